# revision 54
# baseline (speedup 1.0000x reference)
"""Trainium2 Bass kernel for nn_EntailmentSelfAttention (8-core data parallel).

Problem (per batch element n, sentence s):
  q/k/v head projections (shared per-head weights), energy = q @ k.T per head,
  query-position masking, softmax over the QUERY axis, out = attn @ v,
  fc_out: out @ Wo.T + bo.

Design (one batch element n per NeuronCore; S=2 sentences inside):
  - Transposed on-chip layout: head-dim on partitions, sequence on the free
    axis, so the softmax-over-queries reduces along the free axis.
  - q projection folded on the host (yq = q @ Wq^T Wk, compacted+padded);
    v projection folded into fc_out on the host (wcomb); fc bias on host.
  - Masked queries dropped by host compaction; pad columns carry yq = 0 so
    exp(0) = 1, and the denominator subtracts (QP - cnt) (per-core input).
  - ScalarE does ONLY the 64 exp activations (the ~48us pacer engine);
    measured: ANY extra Scalar-queue work (evacs, casts) delays every exp
    behind it and loses more than it saves.
  - Rowsum: TT1+TT2 bf16 halvings + f32 reduce on DVE (GpSimd TT measured
    1.2-1.7us each - too slow for this chain); v-scaling mult on GpSimd.
  - fc: each jt accumulates in two 4-eo halves.  Half-bursts (2 jt x 4 eo
    = 8 matmuls) are sprinkled into later slots as soon as the needed ZT
    pairs are cast (attn@v lags its pair by 2 slots; lag 1 measured worse),
    keeping the PE dense (HAM clock at full rate) and shrinking the
    end-of-kernel tail to s1's second half only.  The two f16 halves are
    DMA'd out separately and summed on the host.
  - PSUM budget exactly 8 banks: energy ring 2x[128,2,512] (4) + z pair
    tile (2) + fc burst tile (2).  The tail reuses energy+z+fc tiles to
    open 8 accumulator banks at once for the final 32 fc matmuls.
  - Row/col-tiled matmul pairs (64-row stationaries at base partitions
    0/64) run concurrently on the PE, so energy and attn@v stream ~2
    cols/cycle in aggregate.
  - DMA: first key/query block split per-pair across sync+scalar queues so
    the pipeline head starts as early as possible; values sliced per block
    on the gpsimd queue; wcomb (2MB) gated behind pair 0's rowsum so the
    round-robin DMA packet scheduler can't starve the pipeline head.
"""

import math

import numpy as np

import concourse.bass as bass
import concourse.tile as tile
from concourse import bacc, mybir
from concourse import bass_utils

# problem shapes (hardcoded per the harness contract)
N, S, L, E, H = 8, 2, 512, 1024, 16
D = E // H  # 64
P = 128
NCORES = 8
LC = L // P  # 4 l-chunks
NP = H // 2  # 8 head pairs
SCALE = 1.0 / math.sqrt(float(L))

F32 = mybir.dt.float32
BF16 = mybir.dt.bfloat16
F16 = mybir.dt.float16

# --- tunables -------------------------------------------------------------
WARMUP_MM = 9        # dummy matmuls at start: keeps the PE busy through
                     # the DMA ramp; the HAM raise completes during the
                     # energy stream that follows without a gap
DUMMY_MM = 4         # filler matmuls per sparse slot to HOLD the PE duty
                     # cycle >= ~60% so HAM doesn't drop back to half clock
POOL_TT2 = False     # second rowsum halving on GpSimd: measured WORSE
                     # twice (Pool TT ~1.2-1.7us each, chain + queue cost)
POOL_XVS = True      # v-scaling multiply on GpSimd (except tail pair)
TT1_SPLIT_PAIRS = set()  # TT1 DVE/Pool split: measured WORSE (89.3 vs 82.7)
# z-cast pairs deferred to Scalar: measured WORSE (85.5/86.9 vs 82.2/82.7)
# - even input-ready Scalar-queue inserts cost more than the DVE relief
SCALAR_CAST_PAIRS = set()

# burst schedule: (s, p) -> list of (early, src_s, half, jt_even) fc burst
# units (2 jt x 4 eo accumulations + inline DVE evac + DMA).  early=True
# units are emitted mid-slot (their ZT pairs were cast in a PREVIOUS slot);
# late units go after this slot's attnv flush, whose cast they need.
BURSTS = {
    (0, 5): [(False, 0, 0, 0)],
    (0, 6): [(True, 0, 0, 2)],
    (0, 7): [(True, 0, 0, 4)],
    (1, 0): [(True, 0, 0, 6)],
    (1, 2): [(True, 0, 1, 0)],
    (1, 3): [(True, 0, 1, 2)],
    (1, 4): [(True, 0, 1, 4)],
    (1, 5): [(True, 0, 1, 6), (False, 1, 0, 0)],
    (1, 6): [(True, 1, 0, 2), (False, 1, 0, 4)],
    (1, 7): [(True, 1, 0, 6)],
}
# sparse slots -> filler matmul count (HAM duty insurance; 6 in the
# late-s0 window where duty sits at the ~50% sustain threshold and traces
# show the clock gate dropping on bad draws)
DUMMY_SLOTS = {(0, 0): 4, (0, 1): 4, (0, 2): 6, (0, 3): 6, (0, 4): 6,
               (0, 5): 6, (1, 1): 4}
# s1 in-energy-tile duty filler: held the clock but measured 86.2/84.6 vs
# 82.9 mean - the extra PE-queue occupancy costs more than the clock buys
EP_FILLER_SLOTS = set()


def build_kernel_body(tc, outs, ins, QP):
    nc = tc.nc

    def _c(ap):
        return ap if ap.dtype == BF16 else ap.bitcast(BF16)

    xk, yq, xv = _c(ins["xk"]), _c(ins["yq"]), _c(ins["xv"])
    wcomb, padq = _c(ins["wcomb"]), ins["padq"]
    outh = outs["outh"]  # (S, 2, E//P//2, P, 2, QP) f16

    import contextlib

    with contextlib.ExitStack() as ctx:
        ek = ctx.enter_context
        consts = ek(tc.tile_pool(name="consts", bufs=1))
        xvpool = ek(tc.tile_pool(name="xv", bufs=2))
        kqpool = ek(tc.tile_pool(name="kq", bufs=4))
        atpool = ek(tc.tile_pool(name="at", bufs=4))
        rtpool = ek(tc.tile_pool(name="rt", bufs=2))
        xvspool = ek(tc.tile_pool(name="xvs", bufs=3))
        sumpool = ek(tc.tile_pool(name="sums", bufs=3))
        ztpool = ek(tc.tile_pool(name="zt", bufs=2))
        outpool = ek(tc.tile_pool(name="out", bufs=4))
        pp_e = ek(tc.tile_pool(name="pp_e", bufs=2, space="PSUM"))
        pp_z = ek(tc.tile_pool(name="pp_z", bufs=1, space="PSUM"))
        pp_f = ek(tc.tile_pool(name="pp_f", bufs=1, space="PSUM"))

        padq_sb = consts.tile([P, S], F32, tag="padq")
        wcomb_sb = consts.tile([P, E // P, E], BF16, tag="wcomb")

        xv_sb, ZT = {}, {}

        # ---- fc burst units: 2 jt x 4-eo accumulations into a 2-bank
        # tile, evacuated inline on the DVE and DMA'd per jt pair ----
        def emit_burst(src_s, half, jta):
            elo = half * 4
            pf = pp_f.tile([P, 2, 512], F32, tag="fc",
                           name=f"fc{src_s}_{half}_{jta}")
            for jt in (jta, jta + 1):
                for eo in range(elo, elo + 4):
                    nc.tensor.matmul(
                        pf[:, jt % 2, 0:QP],
                        wcomb_sb[:, eo, jt * P:(jt + 1) * P],
                        ZT[src_s][:, eo, :],
                        start=(eo == elo),
                        stop=(eo == elo + 3),
                    )
            ot = outpool.tile([P, 2, QP], F16, tag="ot",
                              name=f"ot{src_s}_{half}_{jta}")
            nc.vector.tensor_copy(ot[:], pf[:, :, 0:QP])
            nc.sync.dma_start(outh[src_s, half, jta // 2], ot[:])

        # ---- deferred (software-pipelined) stage emitters ----
        pend_attnv = []
        pend_soft = []
        pend_cast_s = []
        zpair = {}

        def flush_one(q):
            if q:
                q.pop(0)()

        def make_soft(s, p_, at, rsum):
            """Two-stage softmax chain so the Pool-assigned TT2 never sits
            at the head of the DVE queue blocking casts/evacs behind it:
            stage A = TT1 (DVE) + TT2 (Pool), stage B = reduce/den/recip
            (DVE, emitted after this slot's evacs) + xvs (Pool)."""
            holder = {}
            tail = (s == 1 and p_ == NP - 1)
            h1 = QP // 2
            h2 = QP // 4

            def run_a():
                r1 = rtpool.tile([P, LC, 2, h1], BF16, tag="r1",
                                 name=f"r1{s}_{p_}")
                nc.vector.tensor_tensor(
                    r1[:], at[:, :, :, 0:h1], at[:, :, :, h1:QP],
                    mybir.AluOpType.add)
                r2 = rtpool.tile([P, LC, 2, h2], BF16, tag="r2",
                                 name=f"r2{s}_{p_}")
                eng2 = nc.gpsimd if (POOL_TT2 and not tail) else nc.vector
                eng2.tensor_tensor(
                    r2[:], r1[:, :, :, 0:h2], r1[:, :, :, h2:h1],
                    mybir.AluOpType.add)
                holder["r2"] = r2

            def run_b():
                r2 = holder["r2"]
                nc.vector.tensor_reduce(
                    rsum[:].rearrange("p (c i) -> p c i", i=2),
                    r2[:],
                    axis=mybir.AxisListType.X,
                    op=mybir.AluOpType.add,
                )
                den = sumpool.tile([P, LC * 2], F32, tag="den", name=f"dn{s}_{p_}")
                nc.vector.tensor_tensor(
                    den[:], rsum[:],
                    padq_sb[:, s:s + 1].to_broadcast((P, LC * 2)),
                    mybir.AluOpType.subtract)
                recip = sumpool.tile([P, LC, 2], F32, tag="recip",
                                     name=f"rc{s}_{p_}")
                nc.vector.reciprocal(
                    recip[:].rearrange("p c i -> p (c i)"), den[:])
                xvs = xvspool.tile([P, LC, 2, D], BF16, tag="xvs", name=f"xs{s}_{p_}")
                xv_view = xv_sb[s][:, :, 2 * p_ * D:(2 * p_ + 2) * D].rearrange(
                    "p c (i d) -> p c i d", d=D)
                engx = nc.gpsimd if (POOL_XVS and not tail) else nc.vector
                engx.tensor_tensor(
                    xvs[:], xv_view,
                    recip[:, :, :, None].to_broadcast((P, LC, 2, D)),
                    mybir.AluOpType.mult)
                holder["xvs"] = xvs

            def run():
                run_a()
                run_b()
            run.run_a = run_a
            run.run_b = run_b
            run.holder = holder
            return run

        def make_attnv(s, p_, at, soft):
            def run():
                if p_ % 2 == 0:
                    zpair[s] = pp_z.tile([P, 2, 512], F32, tag="z",
                                         name=f"zp{s}_{p_}")
                zp = zpair[s]
                xvs = soft.holder["xvs"]
                for c in range(LC):
                    for i in range(2):
                        nc.tensor.matmul(
                            zp[i * D:(i + 1) * D, p_ % 2, 0:QP],
                            xvs[:, c, i],
                            at[:, c, i],
                            start=(c == 0),
                            stop=(c == LC - 1),
                            skip_group_check=True,
                        )
                # ZT casts: per 2 pairs, except s1 pairs 6/7 are cast
                # singly so the tail's 8-bank fc reuse isn't gated on the
                # whole last z generation.
                if s == 1 and p_ >= NP - 2:
                    nc.vector.tensor_copy(
                        ZT[s][:, p_:p_ + 1, :], zp[:, p_ % 2:p_ % 2 + 1, 0:QP])
                elif p_ % 2 == 1:
                    if (s, p_) in SCALAR_CAST_PAIRS:
                        def scast(zp=zp, s=s, p_=p_):
                            nc.scalar.copy(
                                ZT[s][:, p_ - 1:p_ + 1, :], zp[:, :, 0:QP])
                        pend_cast_s.append(scast)
                    else:
                        nc.vector.tensor_copy(
                            ZT[s][:, p_ - 1:p_ + 1, :], zp[:, :, 0:QP])
            return run

        # ---- DMA block prefetch.  Block 0 (pair 0 alone) is split across
        # the sync + scalar queues so the very first energy matmul can
        # start as early as possible. ----
        blocks = [(0, 0, 1), (0, 1, 1), (0, 2, 2), (0, 4, 2), (0, 6, 2),
                  (1, 0, 2), (1, 2, 2), (1, 4, 2), (1, 6, 2)]
        # block consumed by pair (s,p):
        blk_of = {}
        for bi, (bs, lo, n) in enumerate(blocks):
            for t in range(n):
                blk_of[(bs, lo + t)] = bi
        kq_tiles = {}

        def issue_xv(bi):
            s, lo, n = blocks[bi]
            c0, c1 = 2 * lo * D, 2 * (lo + n) * D
            nc.sync.dma_start(xv_sb[s][:, :, c0:c1], xv[s, :, :, c0:c1])

        def issue_block(bi, yq_eng=None, skip_xv=False):
            if bi >= len(blocks):
                return
            s, lo, n = blocks[bi]
            xkt = kqpool.tile([P, n, L], BF16, tag=f"xk{n}", name=f"xk{s}_{lo}")
            nc.sync.dma_start(xkt[:], xk[s, lo:lo + n].rearrange("t p l -> p t l"))
            yqt = kqpool.tile([P, n, QP], BF16, tag=f"yq{n}", name=f"yq{s}_{lo}")
            (yq_eng or nc.sync).dma_start(
                yqt[:], yq[s, lo:lo + n].rearrange("t p l -> p t l"))
            if not skip_xv:
                issue_xv(bi)
            for t in range(n):
                kq_tiles[(s, lo + t)] = (xkt, yqt, t)

        # ---- main schedule ----
        rsum_gate = [None]
        xv_sb[0] = xvpool.tile([P, LC, E], BF16, tag="xv0", name="xv_0")
        xv_sb[1] = xvpool.tile([P, LC, E], BF16, tag="xv1", name="xv_1")
        # wsrc first, on the otherwise-idle DVE, so the warm-up matmuls can
        # start the moment the PE's program is loaded
        wsrc = consts.tile([P, 512], BF16, tag="wsrc")
        nc.vector.memset(wsrc[:], 0.0)
        issue_block(0, yq_eng=nc.scalar, skip_xv=True)
        issue_block(1, skip_xv=True)
        nc.sync.dma_start(padq_sb[:], padq[:])
        issue_xv(0)
        issue_xv(1)
        issued = [2]

        # prime the exp ACT table load (after the startup DMA issues so the
        # ~1.6us table load doesn't delay block 0's yq on the scalar queue)
        prim = consts.tile([P, 2], F32, tag="prim")
        nc.vector.memset(prim[:, 0:1], 0.0)
        nc.scalar.activation(prim[:, 1:2], prim[:, 0:1],
                             mybir.ActivationFunctionType.Exp)

        # PE warm-up burst during the DMA ramp (lifts the HAM clock gate)
        if WARMUP_MM:
            pwu = pp_e.tile([P, 2, 512], F32, tag="ep", name="ep_warm")
            for w in range(WARMUP_MM):
                nc.tensor.matmul(pwu[:, w % 2, :], wsrc[0:P, 0:P],
                                 wsrc[:], start=True, stop=True)

        def prefetch(s, p_):
            # 3 blocks ahead: rides out transient DMA-bandwidth contention
            # from chip co-tenants (observed +13-19us on starved runs)
            tgt = min(blk_of[(s, p_)] + 3, len(blocks) - 1)
            while issued[0] <= tgt:
                issue_block(issued[0])
                issued[0] += 1

        dummy_state = {}

        def emit_dummies(s, p_):
            # filler matmuls that keep the PE duty cycle high enough for the
            # HAM clock gate to stay at full rate through sparse slots.
            # Targets rotate through psum generations that are dead anyway.
            n_d = DUMMY_SLOTS.get((s, p_), 0)
            if not n_d or not DUMMY_MM:
                return
            if (s, p_) == (0, 0):
                dummy_state["t"] = pp_z.tile([P, 2, 512], F32, tag="z",
                                             name="dz")[:, 0, :]
            elif (s, p_) in ((0, 2), (1, 1)):
                dummy_state["t"] = pp_f.tile([P, 2, 512], F32, tag="fc",
                                             name=f"df{s}_{p_}")[:, 0, :]
            dt_ = dummy_state["t"]
            for w in range(n_d):
                nc.tensor.matmul(dt_, wsrc[0:P, 0:P], wsrc[:],
                                 start=True, stop=True)

        for s in range(S):
            ZT[s] = ztpool.tile([P, NP, QP], BF16, tag=f"zt{s}", name=f"zt_{s}")

            for p_ in range(NP):
                if s == 0 and p_ == 2:
                    # big constant load gated behind a REAL data dependency
                    # (pair 0's rowsum) so the DMA scheduler can't front-run
                    # the key/query stream with it.
                    nc.gpsimd.tensor_copy(wcomb_sb[:, 0, 0:1],
                                          rsum_gate[0][:, 0:1])
                    nc.sync.dma_start(wcomb_sb[:], wcomb[:])
                prefetch(s, p_)
                units = BURSTS.get((s, p_), [])
                early = [u for u in units if u[0]]
                late = [u for u in units if not u[0]]

                # pipelined softmax chain from the previous pair: stage A
                # (TT1 -> Pool TT2) first on the DVE queue, then last slot's
                # fc evacs, then stage B (reduce/den/recip -> Pool xvs) so
                # the reduce never waits at the DVE queue head for Pool.
                if len(pend_soft) >= 1:
                    flush_one(pend_soft)

                xkt, yqt, t = kq_tiles[(s, p_)]
                xk_p, yq_p = xkt[:, t], yqt[:, t]

                at = atpool.tile([P, LC, 2, QP], BF16, tag="at", name=f"at{s}_{p_}")
                rsum = sumpool.tile([P, LC * 2], F32, tag="rsum", name=f"rs{s}_{p_}")
                if s == 0 and p_ == 0:
                    rsum_gate[0] = rsum

                # PE energies + ScalarE exp, chunk by chunk
                for c in range(LC):
                    ep = pp_e.tile([P, 2, 512], F32, tag="ep",
                                   name=f"ep{s}_{p_}_{c}")
                    for i in range(2):
                        nc.tensor.matmul(
                            ep[:, i, 0:QP],
                            xk_p[i * D:(i + 1) * D, c * P:(c + 1) * P],
                            yq_p[i * D:(i + 1) * D, :],
                            start=True,
                            stop=True,
                        )
                    nc.scalar.activation(
                        at[:, c], ep[:, :, 0:QP],
                        mybir.ActivationFunctionType.Exp, scale=SCALE)
                    if c == 1:
                        for u in early:
                            emit_burst(*u[1:])
                    elif c == 2:
                        while pend_cast_s:
                            pend_cast_s.pop(0)()

                emit_dummies(s, p_)
                pend_soft.append(make_soft(s, p_, at, rsum))

                # attn@v from two pairs back, then this slot's late burst
                pend_attnv.append(make_attnv(s, p_, at, pend_soft[-1]))
                if len(pend_attnv) > 2:
                    flush_one(pend_attnv)
                for u in late:
                    emit_burst(*u[1:])

        # ---- tail: attnv(s1,6/7), then s1's fc half 1 (eo 4-7) with all
        # 8 accumulator banks (reusing energy + z + fc psum tiles) ----
        te0 = pp_e.tile([P, 2, 512], F32, tag="ep", name="tl_e0")
        te1 = pp_e.tile([P, 2, 512], F32, tag="ep", name="tl_e1")
        tview = {0: te0[:, 0, 0:QP], 1: te0[:, 1, 0:QP],
                 2: te1[:, 0, 0:QP], 3: te1[:, 1, 0:QP]}

        def tailmm(jt, eo, start, stop):
            nc.tensor.matmul(
                tview[jt],
                wcomb_sb[:, eo, jt * P:(jt + 1) * P],
                ZT[1][:, eo, :],
                start=start, stop=stop, skip_group_check=True)

        for eo in (4, 5):
            for jt in range(4):
                tailmm(jt, eo, eo == 4, False)
        flush_one(pend_attnv)          # attnv(s1,6) + cast(6)
        # jts 6,7 ride the fc-burst tile (free after the last unit's evac,
        # well before cast(7)) so only jts 4,5 wait for the z tile behind
        # cast(7) -> post-cast(7) PE work drops to 14 matmuls
        tf = pp_f.tile([P, 2, 512], F32, tag="fc", name="tl_f")
        tview[6] = tf[:, 0, 0:QP]
        tview[7] = tf[:, 1, 0:QP]
        for jt in range(4):
            tailmm(jt, 6, False, False)
        for eo in (4, 5, 6):
            for jt in (6, 7):
                tailmm(jt, eo, eo == 4, False)
        flush_one(pend_soft)           # soft(s1,7) on DVE
        flush_one(pend_attnv)          # attnv(s1,7) + cast(7)
        tz = pp_z.tile([P, 2, 512], F32, tag="z", name="tl_z")
        tview[4] = tz[:, 0, 0:QP]
        tview[5] = tz[:, 1, 0:QP]
        for eo in (4, 5, 6):
            for jt in (4, 5):
                tailmm(jt, eo, eo == 4, False)
        # final eo: evacuate each 2-jt tile the moment its accumulation
        # stops, overlapping the copies/DMAs with the remaining matmuls
        pair_src = [te0, te1, tz, tf]

        def tail_evac(tt):
            ot = outpool.tile([P, 2, QP], F16, tag="ot", name=f"ot_tl{tt}")
            if tt % 2 == 0:
                nc.vector.tensor_copy(ot[:], pair_src[tt][:, :, 0:QP])
            else:
                nc.scalar.copy(ot[:], pair_src[tt][:, :, 0:QP])
            # fan the four final DMAs across four queues (all engines are
            # idle by now): serial sync-queue issues were stretching the
            # finish ~6us past the last compute op
            dma_eng = (nc.sync, nc.scalar, nc.gpsimd, nc.sync)[tt]
            dma_eng.dma_start(outh[1, 1, tt], ot[:])

        for jt in range(8):
            tailmm(jt, 7, False, True)
            if jt % 2 == 1:
                tail_evac(jt // 2)


def host_prepare(values, keys, query, mask, Wv, Wk, Wq, Wo, bo):
    """Host-side sharding + layout + query compaction + weight folding."""
    values = np.asarray(values, dtype=np.float32)
    keys = np.asarray(keys, dtype=np.float32)
    query = np.asarray(query, dtype=np.float32)
    mask = np.asarray(mask)
    Wv = np.asarray(Wv, dtype=np.float32)
    Wk = np.asarray(Wk, dtype=np.float32)
    Wq = np.asarray(Wq, dtype=np.float32)
    Wo = np.asarray(Wo, dtype=np.float32)
    bo_np = np.ascontiguousarray(np.asarray(bo, dtype=np.float32))

    keep = mask[:, :, :, 0] != 0  # (N, S, L) True = query position survives
    cnt = keep.sum(-1)  # (N, S)
    # multiple of 4 so the two rowsum-tree halvings stay element-aligned
    QP = int(np.ceil(max(int(cnt.max()), 32) / 4) * 4)
    QP = min(QP, L)
    order = np.argsort(~keep, axis=-1, kind="stable")  # (N, S, L)

    qT = query.transpose(0, 1, 3, 2).reshape(N, S, H, D, L)
    kT = keys.transpose(0, 1, 3, 2).reshape(N, S, H, D, L)

    # gather+pad queries: (N, S, H, D, QP)
    gidx = order[:, :, :QP]  # (N, S, QP)
    qTc = np.take_along_axis(
        qT, gidx[:, :, None, None, :].repeat(H, 2).repeat(D, 3), axis=4)
    pad = np.arange(QP)[None, None, :] >= cnt[:, :, None]  # (N, S, QP)
    qTc[pad[:, :, None, None, :].repeat(H, 2).repeat(D, 3)] = 0.0

    # host q-projection: energy[q,k] = (xq A) . xk with A = Wq^T Wk
    A_T = (Wq.T @ Wk).T.copy()  # (D, D)
    yq = np.einsum("de,nshel->nshdl", A_T, qTc)  # (N, S, H, D, QP)
    yq = np.ascontiguousarray(yq.reshape(N, S, NP, 2 * D, QP))
    xkp = np.ascontiguousarray(kT.reshape(N, S, NP, 2 * D, L))

    # values pre-arranged [p, lc, e] with l = lc*128 + p
    xvp = np.ascontiguousarray(
        values.reshape(N, S, LC, P, E).transpose(0, 1, 3, 2, 4))

    wcomb = np.zeros((E, E), np.float32)
    for h in range(H):
        wcomb[h * D:(h + 1) * D, :] = Wv.T @ Wo[:, h * D:(h + 1) * D].T
    wcombp = np.ascontiguousarray(
        wcomb.reshape(E // P, P, E).transpose(1, 0, 2))

    # (N, 128, S): per-core pad-column count, replicated over partitions
    padq = np.repeat((QP - cnt).astype(np.float32)[:, None, :], P, axis=1)
    padq = np.ascontiguousarray(padq)

    import ml_dtypes
    bf = ml_dtypes.bfloat16
    yq = np.ascontiguousarray(yq.astype(bf))
    xkp = np.ascontiguousarray(xkp.astype(bf))
    xvp = np.ascontiguousarray(xvp.astype(bf))
    wcombp = np.ascontiguousarray(wcombp.astype(bf))

    in_maps = []
    for n in range(NCORES):
        m = {
            "yq": yq[n], "xk": xkp[n], "xv": xvp[n],
            "wcomb": wcombp, "padq": padq[n],
        }
        in_maps.append(m)
    return in_maps, QP, order, cnt, bo_np


_NC_CACHE = {}


def _get_program(QP):
    nc = _NC_CACHE.get(QP)
    if nc is not None:
        return nc
    nc = bacc.Bacc("TRN2", target_bir_lowering=False, debug=False,
                   num_devices=NCORES)
    ins = {
        "yq": nc.dram_tensor("yq", (S, NP, P, QP), BF16, kind="ExternalInput").ap(),
        "xk": nc.dram_tensor("xk", (S, NP, P, L), BF16, kind="ExternalInput").ap(),
        "xv": nc.dram_tensor("xv", (S, P, LC, E), BF16, kind="ExternalInput").ap(),
        "wcomb": nc.dram_tensor("wcomb", (P, E // P, E), BF16, kind="ExternalInput").ap(),
        "padq": nc.dram_tensor("padq", (P, S), F32, kind="ExternalInput").ap(),
    }
    outs = {
        "outh": nc.dram_tensor("outh", (S, 2, E // P // 2, P, 2, QP), F16,
                               kind="ExternalOutput").ap(),
    }
    with tile.TileContext(nc) as tc:
        build_kernel_body(tc, outs, ins, QP)
    nc.compile()
    _NC_CACHE[QP] = nc
    return nc


def run(inputs: dict, trace: bool = False):
    """Run on 8 cores; returns (full_output, BassKernelResults)."""
    in_maps, QP, order, cnt, bo_np = host_prepare(**inputs)
    nc = _get_program(QP)
    res = bass_utils.run_bass_kernel_spmd(
        nc, in_maps, core_ids=list(range(NCORES)), trace=trace,
    )
    out = np.empty((N, S, L, E), np.float32)
    out[:] = bo_np  # masked query rows: attention output is 0, fc adds bo
    for n in range(NCORES):
        oh = np.asarray(res.results[n]["outh"], dtype=np.float32)
        # (S, 2, 4, P, 2, QP): halves summed; e = (jp*2 + k)*128 + p
        for s in range(S):
            c = int(cnt[n, s])
            if c:
                full = (oh[s, 0] + oh[s, 1]).transpose(0, 2, 1, 3).reshape(E, QP)
                out[n, s, order[n, s, :c], :] = full[:, :c].T + bo_np
    return out, res


def kernel(**inputs) -> np.ndarray:
    out, _ = run(inputs, trace=False)
    return out
